# revision 14
# baseline (speedup 1.0000x reference)
"""Self-contained TRN2 Bass kernel for the GCN message-passing problem.

8-core SPMD, dst-sharded nodes. v3 design:

- Gather table t[v] (bf16, node-major [N,64]) replicated in every core's
  DRAM via AllGather each layer.
- Edges of a core grouped by dst block (128 dsts), sorted by src within a
  block, padded to tiles of 128 edges. One indirect DMA per tile ([128, 64]
  bf16 rows); tiles are issued to the 4 SWDGE queues in contiguous segments
  of QSEG tiles (per-instruction queue rotation serializes the Q7 cpu
  pairs; segmented issue lets the 4 queue rings drain in parallel).
- One-hot selection matrices are PURE 0/1 (bf16), built with a single DVE
  tensor_tensor is_equal per block over a [128, tbb, 128] broadcast view.
- GCN normalization algebra: table rows are p = dinv*h, psum accumulates the
  UNNORMALIZED segment-sum S = sum_src p (+ self-loop via persistent
  node-major shard in SBUF and an identity rhs). Using
  relu(dinv*y + b) = dinv * relu(y + b*sqrt(deg)), the weight matmul is
  augmented with a 65th contraction row (lhsT row = bias, rhs row = sqrt(deg))
  so h' = dinv * relu(W^T S + b*rdinv) needs NO per-edge scaling; the dinv
  factors fold into the per-partition scaled copies at table write (dinv^2)
  and pooling (dinv).
- Weight matmuls in fp32r (1 cycle/row at >=256 free), message matmuls bf16.
- Pooling: one-hot over G graphs into psum [64, G], AllReduce, tiny MLP.

Measured facts (HW probes, this session) for future optimization attempts:
- indirect_dma_start consumes ONE offset per partition: a multi-column
  offset AP [128, T] does NOT batch T tiles (HW gathers T*64 contiguous
  elems from idx[p,0]; CoreSim's interp semantics diverge from HW here).
- gpsimd.dma_gather works (layout out[i%128, i//128, :] = table[idx[i]],
  idx int16 wrapped [i%16, i//16], elem >= 256B) but num_idxs > 1024
  DEADLOCKS the device (SWDGE ring = 64 descs/engine, not raisable via
  dynamic_dma_scratch_size). Engine-side cost ~4.8us per 1024-idx
  instruction => ~4.7ns/desc with 4 segmented queues; ~7.2ns single-queue;
  per-desc cost is flat in elem size (256B vs 512B) => desc-rate-bound.
- Every fine-grained gather path (SWDGE memcpy ~8ns/desc, dma_gather
  ~4.7ns/desc, TensorE one-hot streaming ~5ns/edge at this density)
  lands at ~5-8ns/edge-row; the conv gathers (~200k edges/core/layer)
  are the hard wall at ~1.2-1.8ms/layer.
- Per-instruction QUEUE ROTATION on SWDGE is pathological (4.7x slower
  than single queue for dma_gather); contiguous segments of ~16
  instructions per queue give the 1.5x multi-queue win (the Ant ucode
  binds queue k to Q7 cpu pair k).
- gpool bufs=96 triggered a walrus CompilerInternalError; 48 compiles.
"""
import time
from dataclasses import dataclass

import numpy as np
import ml_dtypes
import jax
from jax.sharding import Mesh, PartitionSpec
from jax.experimental.shard_map import shard_map

from concourse import bass2jax
from concourse.bass2jax import _bass_exec_p, install_neuronx_cc_hook

import concourse.bass as bass
import concourse.bacc as bacc
import concourse.mybir as mybir
import concourse.tile as tile

F32 = mybir.dt.float32
F32R = mybir.dt.float32r
BF16 = mybir.dt.bfloat16
I32 = mybir.dt.int32
BF = ml_dtypes.bfloat16


@dataclass
class Meta:
    N: int
    F: int
    H: int
    G: int
    L: int
    C: int
    NS: int
    NB: int
    tbb: tuple
    toff: tuple
    T_tot: int
    TBBMAX: int
    CH: int = 512
    BG: int = 4


def preprocess(x, edge_index, batch, W_emb, b_emb, conv_W, conv_b,
               W1, b1, W2, b2, W3, b3, n_cores=8, G=None):
    """Host-side index preprocessing. Returns (meta, in_maps)."""
    x = np.asarray(x, np.float32)
    ei = np.asarray(edge_index, np.int64)
    batch = np.asarray(batch, np.int64)
    N, F = x.shape
    H = int(np.asarray(W_emb).shape[1])
    L = int(np.asarray(conv_W).shape[0])
    C = n_cores
    assert N % C == 0
    NS = N // C
    NB = (NS + 127) // 128
    if G is None:
        G = int(batch.max()) + 1 if batch.size else 1

    loop = np.arange(N, dtype=np.int64)
    deg = (np.bincount(np.concatenate([ei[1], loop]), minlength=N)
           .astype(np.float64))
    dinv = (1.0 / np.sqrt(np.maximum(deg, 1.0))).astype(np.float32)
    rdinv = np.sqrt(np.maximum(deg, 1.0)).astype(np.float32)
    # self-loops are handled as structured tiles (SBUF-resident shard),
    # only the real edges go through indirect gathers
    src, dst = ei[0], ei[1]

    # sort edges by (core, block, src)
    core = dst // NS
    block = (dst % NS) // 128
    order = np.lexsort((src, block, core))
    src_s, dst_s = src[order], dst[order]
    core_s, block_s = core[order], block[order]

    cnt = np.zeros((C, NB), np.int64)
    np.add.at(cnt, (core_s, block_s), 1)
    tbb = np.maximum(1, (cnt.max(axis=0) + 127) // 128).astype(np.int64)
    toff = np.zeros(NB + 1, np.int64)
    toff[1:] = np.cumsum(tbb)
    T_tot = int(toff[-1])
    TBBMAX = int(tbb.max())

    idx_all = np.zeros((C, 128, T_tot), np.int32)
    dstloc = np.full((C, 128, T_tot), -1.0, np.float32)

    starts = np.zeros(C * NB, np.int64)
    starts[1:] = np.cumsum(cnt.ravel())[:-1]
    starts = starts.reshape(C, NB)
    for c in range(C):
        for b in range(NB):
            n = int(cnt[c, b])
            if n == 0:
                continue
            s0 = int(starts[c, b])
            e_src = src_s[s0:s0 + n]
            e_dst = dst_s[s0:s0 + n]
            j = np.arange(n)
            t = int(toff[b]) + j // 128
            p = j % 128
            idx_all[c, p, t] = e_src
            dstloc[c, p, t] = (e_dst - c * NS - b * 128).astype(np.float32)
            # pad gather idx with last valid src; dstloc stays -1 -> zero col
            if (n % 128) != 0:
                lastt = int(toff[b]) + (n - 1) // 128
                idx_all[c, (n % 128):, lastt] = e_src[-1]
            for tt in range(int(toff[b]) + (n + 127) // 128, int(toff[b + 1])):
                idx_all[c, :, tt] = e_src[-1]

    xT = np.ascontiguousarray(x.T.astype(np.float32))  # [F, N]

    iota3 = np.tile(np.arange(128, dtype=np.float32),
                    (128, TBBMAX)).astype(BF).reshape(128, TBBMAX, 128)
    iotag = np.tile(np.arange(G, dtype=np.float32), (128, 1)).astype(BF)
    ident_f = np.eye(128, dtype=np.float32)
    ident_bf = np.eye(128, dtype=np.float32).astype(BF)

    cntg = np.bincount(batch, minlength=G).astype(np.float32)
    invc = np.tile((1.0 / np.maximum(cntg, 1.0))[None, :], (64, 1)).astype(np.float32)

    conv_W = np.asarray(conv_W, np.float32)
    conv_b = np.asarray(conv_b, np.float32)

    meta = Meta(N=N, F=F, H=H, G=G, L=L, C=C, NS=NS, NB=NB,
                tbb=tuple(int(v) for v in tbb),
                toff=tuple(int(v) for v in toff), T_tot=T_tot,
                TBBMAX=TBBMAX)

    in_maps = []
    for c in range(C):
        base = c * NS
        dinv_sh = dinv[base:base + NS]
        dinv_nm = np.zeros((128, NB), np.float32)
        dinv2_nm = np.zeros((128, NB), np.float32)
        poolid = np.full((128, NB), -1.0, np.float32)
        for b in range(NB):
            w = min(128, NS - b * 128)
            dinv_nm[:w, b] = dinv_sh[b * 128:b * 128 + w]
            dinv2_nm[:w, b] = dinv_sh[b * 128:b * 128 + w] ** 2
            poolid[:w, b] = batch[base + b * 128: base + b * 128 + w]
        m = {
            "x_t": np.ascontiguousarray(xT[:, base:base + NS]),
            "idx_all": np.ascontiguousarray(idx_all[c]),
            "dstloc": np.ascontiguousarray(dstloc[c]).astype(BF),
            "poolid": poolid,
            "dinv_nm": dinv_nm,
            "dinv2_nm": dinv2_nm,
            "rdinv": rdinv[base:base + NS].reshape(1, NS),
            "iota3": iota3,
            "iotag": iotag,
            "ident_f": ident_f,
            "ident_bf": ident_bf,
            "wemb": np.asarray(W_emb, np.float32),
            "bemb": np.asarray(b_emb, np.float32).reshape(H, 1),
            "invc": invc,
            "w1": np.asarray(W1, np.float32),
            "b1": np.asarray(b1, np.float32).reshape(-1, 1),
            "w2": np.asarray(W2, np.float32),
            "b2": np.asarray(b2, np.float32).reshape(-1, 1),
            "w3": np.asarray(W3, np.float32),
            "b3": np.asarray(b3, np.float32).reshape(1, 1),
        }
        for i in range(L):
            # augmented weight: row 64 = bias (pairs with rdinv rhs row)
            m[f"cwa{i}"] = np.ascontiguousarray(
                np.concatenate([conv_W[i], conv_b[i][None, :]], axis=0))
        in_maps.append(m)
    return meta, in_maps


def build_nc(meta: Meta, repeats=1, variant="full", NQ=4):
    N, F, H, G, L, C = meta.N, meta.F, meta.H, meta.G, meta.L, meta.C
    NS, NB, CH, BG = meta.NS, meta.NB, meta.CH, meta.BG
    tbb, toff, T_tot, TBBMAX = meta.tbb, meta.toff, meta.T_tot, meta.TBBMAX
    NCH = (NS + CH - 1) // CH

    nc = bacc.Bacc("TRN2", target_bir_lowering=False, debug=False, num_devices=C,
                   num_swdge_queues=NQ)

    def EIN(name, shape, dt):
        return nc.dram_tensor(name, list(shape), dt, kind="ExternalInput")

    x_t = EIN("x_t", [F, NS], F32)
    idx_all = EIN("idx_all", [128, T_tot], I32)
    dstloc = EIN("dstloc", [128, T_tot], BF16)
    poolid = EIN("poolid", [128, NB], F32)
    dinv_nm = EIN("dinv_nm", [128, NB], F32)
    dinv2_nm = EIN("dinv2_nm", [128, NB], F32)
    rdinv = EIN("rdinv", [1, NS], F32)
    iota3 = EIN("iota3", [128, TBBMAX, 128], BF16)
    iotag = EIN("iotag", [128, G], BF16)
    ident_f = EIN("ident_f", [128, 128], F32)
    ident_bf = EIN("ident_bf", [128, 128], BF16)
    wemb = EIN("wemb", [F, H], F32)
    bemb = EIN("bemb", [H, 1], F32)
    invc = EIN("invc", [64, G], F32)
    w1 = EIN("w1", [H, H], F32)
    b1 = EIN("b1", [H, 1], F32)
    w2 = EIN("w2", [H, H // 2], F32)
    b2 = EIN("b2", [H // 2, 1], F32)
    w3 = EIN("w3", [H // 2, 1], F32)
    b3 = EIN("b3", [1, 1], F32)
    cwa = [EIN(f"cwa{i}", [H + 1, H], F32) for i in range(L)]

    out_d = nc.dram_tensor("out", [1, G], F32, kind="ExternalOutput")

    table_a = nc.dram_tensor("table_a", [N, H], BF16, addr_space="Shared")
    table_b = nc.dram_tensor("table_b", [N, H], BF16, addr_space="Shared")
    bounce = nc.dram_tensor("bounce", [NS, H], BF16)
    pool_in = nc.dram_tensor("pool_in", [H, G], F32)
    pool_out = nc.dram_tensor("pool_out", [H, G], F32, addr_space="Shared")

    groups = [list(range(C))]

    # gathers are issued to SWDGE queues in contiguous segments of QSEG
    # tiles; per-instruction queue rotation serializes the Q7 cpu pairs.
    QSEG = 16

    with tile.TileContext(nc) as tc:
        import contextlib
        ctx = contextlib.ExitStack()
        with ctx:
            P = ctx.enter_context
            persist = P(tc.tile_pool(name="persist", bufs=1))
            xpool = P(tc.tile_pool(name="xpool", bufs=3))
            gpool = P(tc.tile_pool(name="gpool", bufs=48))
            ohpool = P(tc.tile_pool(name="ohpool", bufs=6))
            pohpool = P(tc.tile_pool(name="pohpool", bufs=3))
            bp_ps = P(tc.tile_pool(name="bp_ps", bufs=3, space="PSUM"))
            mm_ps = P(tc.tile_pool(name="mm_ps", bufs=2, space="PSUM"))
            tr_ps = P(tc.tile_pool(name="tr_ps", bufs=2, space="PSUM"))

            def load(name, ap, shape, dt):
                t = persist.tile(list(shape), dt, tag=name)
                nc.sync.dma_start(out=t[:], in_=ap[:])
                return t

            idx_sb = load("idx_sb", idx_all, [128, T_tot], I32)
            dstloc_sb = load("dstloc_sb", dstloc, [128, T_tot], BF16)
            poolid_sb = load("poolid_sb", poolid, [128, NB], F32)
            dinvnm_sb = load("dinvnm_sb", dinv_nm, [128, NB], F32)
            dinv2nm_sb = load("dinv2nm_sb", dinv2_nm, [128, NB], F32)
            iota3_sb = load("iota3_sb", iota3, [128, TBBMAX, 128], BF16)
            iotag_sb = load("iotag_sb", iotag, [128, G], BF16)
            identf_sb = load("identf_sb", ident_f, [128, 128], F32)
            identbf_sb = load("identbf_sb", ident_bf, [128, 128], BF16)
            wemb_sb = load("wemb_sb", wemb, [F, H], F32)
            bemb_sb = load("bemb_sb", bemb, [H, 1], F32)
            invc_sb = load("invc_sb", invc, [64, G], F32)
            w1_sb = load("w1_sb", w1, [H, H], F32)
            b1_sb = load("b1_sb", b1, [H, 1], F32)
            w2_sb = load("w2_sb", w2, [H, H // 2], F32)
            b2_sb = load("b2_sb", b2, [H // 2, 1], F32)
            w3_sb = load("w3_sb", w3, [H // 2, 1], F32)
            b3_sb = load("b3_sb", b3, [1, 1], F32)
            cwa_sb = [load(f"cwa{i}_sb", cwa[i], [H + 1, H], F32)
                      for i in range(L)]

            # hagg rows 0:64 = working activation (S or q), row 64 = rdinv
            hagg = persist.tile([H + 1, NS], F32, tag="hagg")
            nc.sync.dma_start(out=hagg[H:H + 1, :], in_=rdinv[:])
            # node-major bf16 copy of this core's scaled shard (self-loops)
            shard_nm = persist.tile([128, NB * H], BF16, tag="shard_nm")
            h3n = persist.tile([128, NB * H], BF16, tag="h3n")

            iota3v = iota3_sb

            def chunks():
                for ci in range(NCH):
                    c0 = ci * CH
                    yield c0, min(CH, NS - c0)

            def table_write(table_out, scale_sb):
                for b in range(NB):
                    w = min(128, NS - b * 128)
                    tp = tr_ps.tile([128, 64], F32, tag="trp")
                    nc.tensor.transpose(
                        out=tp[:w, :], in_=hagg[:H, b * 128:b * 128 + w],
                        identity=identf_sb[:64, :64])
                    nc.vector.tensor_scalar(
                        out=shard_nm[:w, b * H:b * H + H], in0=tp[:w, :],
                        scalar1=scale_sb[:w, b:b + 1], scalar2=None,
                        op0=mybir.AluOpType.mult)
                    nc.sync.dma_start(
                        out=bounce[b * 128:b * 128 + w, :],
                        in_=shard_nm[:w, b * H:b * H + H])
                if variant == "nocoll":
                    nc.sync.dma_start(out=table_out[:NS, :], in_=bounce[:])
                else:
                    nc.gpsimd.collective_compute(
                        "AllGather", mybir.AluOpType.bypass,
                        replica_groups=groups,
                        ins=[bounce[:]], outs=[table_out[:]])

            for _rep in range(repeats):
                # ================= embed =================
                for c0, cwd in chunks():
                    xt = xpool.tile([F, CH], F32, tag="xt")
                    nc.sync.dma_start(out=xt[:, :cwd], in_=x_t[:, c0:c0 + cwd])
                    ps = mm_ps.tile([64, CH], F32, tag="mmps")
                    nc.tensor.matmul(out=ps[:, :cwd], lhsT=wemb_sb[:],
                                     rhs=xt[:, :cwd], start=True, stop=True)
                    nc.scalar.activation(out=hagg[:H, c0:c0 + cwd], in_=ps[:, :cwd],
                                         func=mybir.ActivationFunctionType.Relu,
                                         bias=bemb_sb[:, 0:1])
                table_write(table_a, dinvnm_sb)

                # ================= conv layers =================
                tables = [table_a, table_b, table_a]
                for li in range(L):
                    t_in = tables[li]
                    for b in range(NB):
                        w = min(128, NS - b * 128)
                        k = tbb[b]
                        oh = ohpool.tile([128, TBBMAX, 128], BF16, tag="oh")
                        nc.vector.tensor_tensor(
                            out=oh[:, :k, :], in0=iota3v[:, :k, :],
                            in1=dstloc_sb[:, toff[b]:toff[b] + k]
                            .to_broadcast([128, k, 128]),
                            op=mybir.AluOpType.is_equal)
                        ps = bp_ps.tile([64, 128], F32, tag="bps")
                        first = True
                        for t in range(k):
                            tt = toff[b] + t
                            g = gpool.tile([128, H], BF16, tag="g")
                            gi = nc.gpsimd.indirect_dma_start(
                                out=g[:], out_offset=None, in_=t_in[:],
                                in_offset=bass.IndirectOffsetOnAxis(
                                    ap=idx_sb[:, tt:tt + 1], axis=0))
                            # contiguous queue segments (rotation per-inst
                            # serializes the SWDGE cpu pairs; segments let
                            # the 4 queue rings drain in parallel)
                            qn = (tt // QSEG) % NQ
                            if qn:
                                gi.ins.queue = "qPoolDynamic%d" % qn
                            nc.tensor.matmul(
                                out=ps[:], lhsT=g[:], rhs=oh[:, t, :],
                                start=first, stop=False)
                            first = False
                        # self-loop: own scaled shard rows, identity rhs
                        nc.tensor.matmul(
                            out=ps[:], lhsT=shard_nm[:w, b * H:b * H + H],
                            rhs=identbf_sb[:w, :],
                            start=False, stop=True)
                        nc.scalar.activation(
                            out=hagg[:H, b * 128:b * 128 + w],
                            in_=ps[:, :w],
                            func=mybir.ActivationFunctionType.Copy)
                    for c0, cwd in chunks():
                        ps = mm_ps.tile([64, CH], F32, tag="mmps")
                        nc.tensor.matmul(out=ps[:, :cwd], lhsT=cwa_sb[li][:],
                                         rhs=hagg[:, c0:c0 + cwd],
                                         start=True, stop=True)
                        nc.scalar.activation(out=hagg[:H, c0:c0 + cwd],
                                             in_=ps[:, :cwd],
                                             func=mybir.ActivationFunctionType.Relu)
                    if li < L - 1:
                        table_write(tables[li + 1], dinv2nm_sb)

                # ================= pooling =================
                # h3 = dinv * q3; pooled = segsum_G(h3)
                for b in range(NB):
                    w = min(128, NS - b * 128)
                    tp = tr_ps.tile([128, 64], F32, tag="trp")
                    nc.tensor.transpose(out=tp[:w, :],
                                        in_=hagg[:H, b * 128:b * 128 + w],
                                        identity=identf_sb[:64, :64])
                    nc.vector.tensor_scalar(
                        out=h3n[:w, b * H:b * H + H], in0=tp[:w, :],
                        scalar1=dinvnm_sb[:w, b:b + 1], scalar2=None,
                        op0=mybir.AluOpType.mult)
                with tc.tile_pool(name="pool_ps", bufs=1, space="PSUM") as pool_ps:
                    pps = pool_ps.tile([64, G], F32, tag="pps")
                    for b in range(NB):
                        w = min(128, NS - b * 128)
                        ohp = pohpool.tile([128, G], BF16, tag="ohp")
                        nc.vector.tensor_scalar(
                            out=ohp[:w, :], in0=iotag_sb[:w, :],
                            scalar1=poolid_sb[:w, b:b + 1], scalar2=None,
                            op0=mybir.AluOpType.is_equal)
                        nc.tensor.matmul(out=pps[:],
                                         lhsT=h3n[:w, b * H:b * H + H],
                                         rhs=ohp[:w, :],
                                         start=(b == 0), stop=(b == NB - 1))
                    psum_sb = persist.tile([64, G], F32, tag="psum_sb")
                    nc.vector.tensor_copy(out=psum_sb[:], in_=pps[:])
                nc.sync.dma_start(out=pool_in[:], in_=psum_sb[:])
                nc.gpsimd.collective_compute(
                    "AllReduce", mybir.AluOpType.add, replica_groups=groups,
                    ins=[pool_in[:]], outs=[pool_out[:]])
                pooled = persist.tile([64, G], F32, tag="pooled")
                nc.sync.dma_start(out=pooled[:], in_=pool_out[:])
                nc.vector.tensor_tensor(out=pooled[:], in0=pooled[:], in1=invc_sb[:],
                                        op=mybir.AluOpType.mult)
                # ================= MLP =================
                ps1 = mm_ps.tile([64, CH], F32, tag="mmps")
                nc.tensor.matmul(out=ps1[:, :G], lhsT=w1_sb[:], rhs=pooled[:],
                                 start=True, stop=True)
                r1 = persist.tile([64, G], F32, tag="r1")
                nc.scalar.activation(out=r1[:], in_=ps1[:64, :G],
                                     func=mybir.ActivationFunctionType.Relu,
                                     bias=b1_sb[:, 0:1])
                ps2 = mm_ps.tile([64, CH], F32, tag="mmps")
                nc.tensor.matmul(out=ps2[:32, :G], lhsT=w2_sb[:], rhs=r1[:],
                                 start=True, stop=True)
                r2 = persist.tile([32, G], F32, tag="r2")
                nc.scalar.activation(out=r2[:], in_=ps2[:32, :G],
                                     func=mybir.ActivationFunctionType.Relu,
                                     bias=b2_sb[:, 0:1])
                ps3 = mm_ps.tile([64, CH], F32, tag="mmps")
                nc.tensor.matmul(out=ps3[:1, :G], lhsT=w3_sb[:], rhs=r2[:],
                                 start=True, stop=True)
                outs = persist.tile([1, G], F32, tag="outs")
                nc.vector.tensor_scalar(out=outs[:], in0=ps3[:1, :G],
                                        scalar1=b3_sb[0:1, 0:1], scalar2=None,
                                        op0=mybir.AluOpType.add)
                nc.sync.dma_start(out=out_d[:], in_=outs[:])

    nc.compile()
    return nc


class SpmdRunner:
    def __init__(self, nc, n_cores):
        install_neuronx_cc_hook()
        self.nc = nc
        self.n_cores = n_cores
        partition_name = (nc.partition_id_tensor.name
                          if nc.partition_id_tensor else None)
        in_names, out_names, out_avals, zero_outs = [], [], [], []
        for alloc in nc.m.functions[0].allocations:
            if not isinstance(alloc, mybir.MemoryLocationSet):
                continue
            name = alloc.memorylocations[0].name
            if alloc.kind == "ExternalInput":
                if name != partition_name:
                    in_names.append(name)
            elif alloc.kind == "ExternalOutput":
                shape = tuple(alloc.tensor_shape)
                dt = mybir.dt.np(alloc.dtype)
                out_names.append(name)
                out_avals.append(jax.core.ShapedArray(shape, dt))
                zero_outs.append(np.zeros(shape, dt))
        self.in_names, self.out_names = in_names, out_names
        self.zero_outs = zero_outs
        bind_in_names = in_names + out_names
        if partition_name is not None:
            bind_in_names.append(partition_name)

        def _body(*args):
            operands = list(args)
            if partition_name is not None:
                operands.append(bass2jax.partition_id_tensor())
            outs = _bass_exec_p.bind(
                *operands,
                out_avals=tuple(out_avals),
                in_names=tuple(bind_in_names),
                out_names=tuple(out_names),
                lowering_input_output_aliases=(),
                sim_require_finite=False,
                sim_require_nnan=False,
                nc=nc,
            )
            return tuple(outs)

        devices = jax.devices()[:n_cores]
        self.mesh = Mesh(np.asarray(devices), ("core",))
        n_args = len(in_names) + len(zero_outs)
        in_specs = (PartitionSpec("core"),) * n_args
        out_specs = (PartitionSpec("core"),) * len(out_names)
        self.fn = jax.jit(
            shard_map(_body, mesh=self.mesh, in_specs=in_specs,
                      out_specs=out_specs, check_rep=False),
            keep_unused=True,
        )
        self._dev_in = None

    def set_inputs(self, in_maps):
        assert len(in_maps) == self.n_cores
        concat = [np.concatenate([np.asarray(in_maps[c][n])
                                  for c in range(self.n_cores)], axis=0)
                  for n in self.in_names]
        self._dev_in = [jax.device_put(a) for a in concat]
        self._dev_zeros = [
            jax.device_put(np.zeros((self.n_cores * z.shape[0], *z.shape[1:]),
                                    z.dtype)) for z in self.zero_outs]
        jax.block_until_ready(self._dev_in)

    def run(self):
        outs = self.fn(*self._dev_in, *self._dev_zeros)
        jax.block_until_ready(outs)
        return outs

    def results(self, outs):
        res = [dict() for _ in range(self.n_cores)]
        for i, name in enumerate(self.out_names):
            arr = np.asarray(outs[i])
            per = np.split(arr, self.n_cores, axis=0)
            for c in range(self.n_cores):
                res[c][name] = per[c]
        return res


_CACHE = {}


def _get_runner(meta, in_maps, repeats=1, variant="full", NQ=4):
    key = (tuple(sorted(meta.__dict__.items())), repeats, variant, NQ)
    if key not in _CACHE:
        nc = build_nc(meta, repeats=repeats, variant=variant, NQ=NQ)
        _CACHE[key] = SpmdRunner(nc, meta.C)
    return _CACHE[key]


def kernel(x, edge_index, batch, W_emb, b_emb, conv_W, conv_b,
           W1, b1, W2, b2, W3, b3):
    """Full (unsharded) inputs -> full [G, 1] float32 output."""
    G = 256
    meta, in_maps = preprocess(
        x, edge_index, batch, W_emb, b_emb, conv_W, conv_b,
        W1, b1, W2, b2, W3, b3, n_cores=8, G=G)
    r = _get_runner(meta, in_maps)
    r.set_inputs(in_maps)
    res = r.results(r.run())
    return np.ascontiguousarray(res[0]["out"].reshape(G, 1).astype(np.float32))



# revision 23
# speedup vs baseline: 1.7112x; 1.7112x over previous
"""Self-contained TRN2 Bass kernel for the GCN message-passing problem.

8-core SPMD, dst-sharded nodes. v3 design:

- Gather table t[v] (bf16, node-major [N,64]) replicated in every core's
  DRAM via AllGather each layer.
- Edges of a core grouped by dst block (128 dsts), sorted by src within a
  block, padded to tiles of 128 edges. One indirect DMA per tile ([128, 64]
  bf16 rows); tiles are issued to the 4 SWDGE queues in contiguous segments
  of QSEG tiles (per-instruction queue rotation serializes the Q7 cpu
  pairs; segmented issue lets the 4 queue rings drain in parallel).
- One-hot selection matrices are PURE 0/1 (bf16), built with a single DVE
  tensor_tensor is_equal per block over a [128, tbb, 128] broadcast view.
- GCN normalization algebra: table rows are p = dinv*h, psum accumulates the
  UNNORMALIZED segment-sum S = sum_src p (+ self-loop via persistent
  node-major shard in SBUF and an identity rhs). Using
  relu(dinv*y + b) = dinv * relu(y + b*sqrt(deg)), the weight matmul is
  augmented with a 65th contraction row (lhsT row = bias, rhs row = sqrt(deg))
  so h' = dinv * relu(W^T S + b*rdinv) needs NO per-edge scaling; the dinv
  factors fold into the per-partition scaled copies at table write (dinv^2)
  and pooling (dinv).
- Weight matmuls in fp32r (1 cycle/row at >=256 free), message matmuls bf16.
- Pooling: one-hot over G graphs into psum [64, G], AllReduce, tiny MLP.

Measured facts (HW probes, this session) for future optimization attempts:
- indirect_dma_start consumes ONE offset per partition: a multi-column
  offset AP [128, T] does NOT batch T tiles (HW gathers T*64 contiguous
  elems from idx[p,0]; CoreSim's interp semantics diverge from HW here).
- gpsimd.dma_gather works (layout out[i%128, i//128, :] = table[idx[i]],
  idx int16 wrapped [i%16, i//16], elem >= 256B) but num_idxs > 1024
  DEADLOCKS the device (SWDGE ring = 64 descs/engine, not raisable via
  dynamic_dma_scratch_size). Engine-side cost ~4.8us per 1024-idx
  instruction => ~4.7ns/desc with 4 segmented queues; ~7.2ns single-queue;
  per-desc cost is flat in elem size (256B vs 512B) => desc-rate-bound.
- Every fine-grained gather path (SWDGE memcpy ~8ns/desc, dma_gather
  ~4.7ns/desc, TensorE one-hot streaming ~5ns/edge at this density)
  lands at ~5-8ns/edge-row; the conv gathers (~200k edges/core/layer)
  are the hard wall at ~1.2-1.8ms/layer.
- Per-instruction QUEUE ROTATION on SWDGE is pathological (4.7x slower
  than single queue for dma_gather); contiguous segments of ~16
  instructions per queue give the 1.5x multi-queue win (the Ant ucode
  binds queue k to Q7 cpu pair k).
- gpool bufs=96 triggered a walrus CompilerInternalError; 48 compiles.
"""
import time
from dataclasses import dataclass

import numpy as np
import ml_dtypes
import jax
from jax.sharding import Mesh, PartitionSpec
from jax.experimental.shard_map import shard_map

from concourse import bass2jax
from concourse.bass2jax import _bass_exec_p, install_neuronx_cc_hook

import concourse.bass as bass
import concourse.bacc as bacc
import concourse.mybir as mybir
import concourse.tile as tile

F32 = mybir.dt.float32
F32R = mybir.dt.float32r
BF16 = mybir.dt.bfloat16
I32 = mybir.dt.int32
BF = ml_dtypes.bfloat16


@dataclass
class Meta:
    N: int
    F: int
    H: int
    G: int
    L: int
    C: int
    NS: int
    NB: int
    tcb: tuple        # [NCHK][NB] tiles per (chunk, block)
    toff_cb: tuple    # cumsum over (chunk, block) raveled, len NCHK*NB+1
    instrs: tuple     # ((chunk, t0, t1), ...) gather instructions
    firstk: tuple     # per block: first chunk with tiles (for hagg init)
    T_tot: int
    TBBMAX: int       # max tiles per (chunk, block-group) for iota3/oh
    CS: int = 25000
    NCHK: int = 4
    GB: int = 6
    CH: int = 512


def preprocess(x, edge_index, batch, W_emb, b_emb, conv_W, conv_b,
               W1, b1, W2, b2, W3, b3, n_cores=8, G=None):
    """Host-side index preprocessing. Returns (meta, in_maps)."""
    x = np.asarray(x, np.float32)
    ei = np.asarray(edge_index, np.int64)
    batch = np.asarray(batch, np.int64)
    N, F = x.shape
    H = int(np.asarray(W_emb).shape[1])
    L = int(np.asarray(conv_W).shape[0])
    C = n_cores
    assert N % C == 0
    NS = N // C
    NB = (NS + 127) // 128
    if G is None:
        G = int(batch.max()) + 1 if batch.size else 1

    loop = np.arange(N, dtype=np.int64)
    deg = (np.bincount(np.concatenate([ei[1], loop]), minlength=N)
           .astype(np.float64))
    dinv = (1.0 / np.sqrt(np.maximum(deg, 1.0))).astype(np.float32)
    rdinv = np.sqrt(np.maximum(deg, 1.0)).astype(np.float32)
    # self-loops are handled as structured tiles (SBUF-resident shard),
    # only the real edges go through dma_gather
    src, dst = ei[0], ei[1]

    CS, NCHK, GB = 25000, 4, 6
    # sort edges by (core, chunk, block, src): chunk-major tile order so
    # gather instructions pack to the 1024-idx SWDGE ring limit
    core = dst // NS
    block = (dst % NS) // 128
    chunk = src // CS
    order = np.lexsort((src, block, chunk, core))
    src_s, dst_s = src[order], dst[order]
    core_s, chunk_s, block_s = core[order], chunk[order], block[order]

    cnt = np.zeros((C, NCHK, NB), np.int64)
    np.add.at(cnt, (core_s, chunk_s, block_s), 1)
    tcb = ((cnt.max(axis=0) + 127) // 128).astype(np.int64)  # [NCHK, NB]
    toff_cb = np.zeros(NCHK * NB + 1, np.int64)
    toff_cb[1:] = np.cumsum(tcb.ravel())
    T_tot = int(toff_cb[-1])
    firstk = [min(k for k in range(NCHK) if tcb[k, b] > 0)
              if tcb[:, b].sum() > 0 else NCHK - 1 for b in range(NB)]

    # gather instructions: runs of <=8 consecutive tiles within a chunk
    instrs = []
    for k in range(NCHK):
        t = int(toff_cb[k * NB])
        tend = int(toff_cb[(k + 1) * NB])
        while t < tend:
            t1 = min(t + 8, tend)
            instrs.append((k, t, t1))
            t = t1

    # block groups (per chunk) for one-hot builds
    TBBMAX = 0
    for k in range(NCHK):
        for b0 in range(0, NB, GB):
            b1g = min(b0 + GB, NB)
            ng = int(toff_cb[k * NB + b1g] - toff_cb[k * NB + b0])
            TBBMAX = max(TBBMAX, ng)

    idx16 = np.zeros((C, 128, 8 * T_tot), np.int16)
    dstloc = np.full((C, 128, T_tot), -1.0, np.float32)

    starts = np.zeros(C * NCHK * NB, np.int64)
    starts[1:] = np.cumsum(cnt.ravel())[:-1]
    starts = starts.reshape(C, NCHK, NB)
    for c in range(C):
        for k in range(NCHK):
            for b in range(NB):
                ntil = int(tcb[k, b])
                if ntil == 0:
                    continue
                n = int(cnt[c, k, b])
                t0 = int(toff_cb[k * NB + b])
                s0 = int(starts[c, k, b])
                e_src = src_s[s0:s0 + n] - k * CS   # chunk-local
                e_dst = dst_s[s0:s0 + n]
                fill = int(e_src[-1]) if n else 0
                j = np.arange(ntil * 128)
                t = t0 + j // 128
                p = j % 128
                vals = np.full(ntil * 128, fill, np.int16)
                vals[:n] = e_src.astype(np.int16)
                idx16[c, p % 16, 8 * t + p // 16] = vals
                dstloc[c, p[:n], t[:n]] = (e_dst - c * NS - b * 128
                                           ).astype(np.float32)
        for band in range(1, 8):
            idx16[c, 16 * band:16 * band + 16, :] = idx16[c, 0:16, :]

    xT = np.ascontiguousarray(x.T.astype(np.float32))  # [F, N]

    iota3 = np.tile(np.arange(128, dtype=np.float32),
                    (128, TBBMAX)).astype(BF).reshape(128, TBBMAX, 128)
    iotag = np.tile(np.arange(G, dtype=np.float32), (128, 1)).astype(BF)
    ident_f = np.eye(128, dtype=np.float32)
    ident_bf = np.eye(128, dtype=np.float32).astype(BF)

    cntg = np.bincount(batch, minlength=G).astype(np.float32)
    invc = np.tile((1.0 / np.maximum(cntg, 1.0))[None, :], (64, 1)).astype(np.float32)

    conv_W = np.asarray(conv_W, np.float32)
    conv_b = np.asarray(conv_b, np.float32)

    meta = Meta(N=N, F=F, H=H, G=G, L=L, C=C, NS=NS, NB=NB,
                tcb=tuple(tuple(int(v) for v in row) for row in tcb),
                toff_cb=tuple(int(v) for v in toff_cb),
                instrs=tuple(instrs), firstk=tuple(firstk),
                T_tot=T_tot, TBBMAX=TBBMAX, CS=CS, NCHK=NCHK, GB=GB)

    in_maps = []
    for c in range(C):
        base = c * NS
        dinv_sh = dinv[base:base + NS]
        dinv_nm = np.zeros((128, NB), np.float32)
        dinv2_nm = np.zeros((128, NB), np.float32)
        poolid = np.full((128, NB), -1.0, np.float32)
        for b in range(NB):
            w = min(128, NS - b * 128)
            dinv_nm[:w, b] = dinv_sh[b * 128:b * 128 + w]
            dinv2_nm[:w, b] = dinv_sh[b * 128:b * 128 + w] ** 2
            poolid[:w, b] = batch[base + b * 128: base + b * 128 + w]
        m = {
            "x_t": np.ascontiguousarray(xT[:, base:base + NS]),
            "idx16": np.ascontiguousarray(idx16[c]),
            "dstloc": np.ascontiguousarray(dstloc[c]).astype(BF),
            "poolid": poolid,
            "dinv_nm": dinv_nm,
            "dinv2_nm": dinv2_nm,
            "rdinv": rdinv[base:base + NS].reshape(1, NS),
            "iota3": iota3,
            "iotag": iotag,
            "ident_f": ident_f,
            "ident_bf": ident_bf,
            "wemb": np.asarray(W_emb, np.float32),
            "bemb": np.asarray(b_emb, np.float32).reshape(H, 1),
            "invc": invc,
            "w1": np.asarray(W1, np.float32),
            "b1": np.asarray(b1, np.float32).reshape(-1, 1),
            "w2": np.asarray(W2, np.float32),
            "b2": np.asarray(b2, np.float32).reshape(-1, 1),
            "w3": np.asarray(W3, np.float32),
            "b3": np.asarray(b3, np.float32).reshape(1, 1),
        }
        for i in range(L):
            # augmented weight: row 64 = bias (pairs with rdinv rhs row)
            m[f"cwa{i}"] = np.ascontiguousarray(
                np.concatenate([conv_W[i], conv_b[i][None, :]], axis=0))
        in_maps.append(m)
    return meta, in_maps


def build_nc(meta: Meta, repeats=1, variant="full", NQ=4):
    N, F, H, G, L, C = meta.N, meta.F, meta.H, meta.G, meta.L, meta.C
    NS, NB, CH = meta.NS, meta.NB, meta.CH
    T_tot, TBBMAX = meta.T_tot, meta.TBBMAX
    tcb, toff_cb, instrs = meta.tcb, meta.toff_cb, meta.instrs
    firstk, CS, NCHK, GB = meta.firstk, meta.CS, meta.NCHK, meta.GB
    NCH = (NS + CH - 1) // CH
    I16 = mybir.dt.int16

    nc = bacc.Bacc("TRN2", target_bir_lowering=False, debug=False, num_devices=C,
                   num_swdge_queues=NQ)

    def EIN(name, shape, dt):
        return nc.dram_tensor(name, list(shape), dt, kind="ExternalInput")

    x_t = EIN("x_t", [F, NS], F32)
    idx16 = EIN("idx16", [128, 8 * T_tot], I16)
    dstloc = EIN("dstloc", [128, T_tot], BF16)
    poolid = EIN("poolid", [128, NB], F32)
    dinv_nm = EIN("dinv_nm", [128, NB], F32)
    dinv2_nm = EIN("dinv2_nm", [128, NB], F32)
    rdinv = EIN("rdinv", [1, NS], F32)
    iota3 = EIN("iota3", [128, TBBMAX, 128], BF16)
    iotag = EIN("iotag", [128, G], BF16)
    ident_f = EIN("ident_f", [128, 128], F32)
    ident_bf = EIN("ident_bf", [128, 128], BF16)
    wemb = EIN("wemb", [F, H], F32)
    bemb = EIN("bemb", [H, 1], F32)
    invc = EIN("invc", [64, G], F32)
    w1 = EIN("w1", [H, H], F32)
    b1 = EIN("b1", [H, 1], F32)
    w2 = EIN("w2", [H, H // 2], F32)
    b2 = EIN("b2", [H // 2, 1], F32)
    w3 = EIN("w3", [H // 2, 1], F32)
    b3 = EIN("b3", [1, 1], F32)
    cwa = [EIN(f"cwa{i}", [H + 1, H], F32) for i in range(L)]

    out_d = nc.dram_tensor("out", [1, G], F32, kind="ExternalOutput")

    # table rows padded to 128 cols (256B) for dma_gather's min elem size;
    # cols 64:128 are never read (lhsT slices 0:64)
    table_a = nc.dram_tensor("table_a", [N, 128], BF16, addr_space="Shared")
    table_b = nc.dram_tensor("table_b", [N, 128], BF16, addr_space="Shared")
    bounce = nc.dram_tensor("bounce", [NS, 128], BF16)
    pool_in = nc.dram_tensor("pool_in", [H, G], F32)
    pool_out = nc.dram_tensor("pool_out", [H, G], F32, addr_space="Shared")

    groups = [list(range(C))]

    # gather instructions go to SWDGE queues in contiguous segments
    # (per-instruction rotation serializes the Q7 cpu pairs)
    QSEG = 8

    with tile.TileContext(nc) as tc:
        import contextlib
        ctx = contextlib.ExitStack()
        with ctx:
            P = ctx.enter_context
            persist = P(tc.tile_pool(name="persist", bufs=1))
            xpool = P(tc.tile_pool(name="xpool", bufs=3))
            gpool = P(tc.tile_pool(name="gpool", bufs=16))
            ohpool = P(tc.tile_pool(name="ohpool", bufs=4))
            pohpool = P(tc.tile_pool(name="pohpool", bufs=3))
            bp_ps = P(tc.tile_pool(name="bp_ps", bufs=3, space="PSUM"))
            mm_ps = P(tc.tile_pool(name="mm_ps", bufs=2, space="PSUM"))
            tr_ps = P(tc.tile_pool(name="tr_ps", bufs=2, space="PSUM"))

            def load(name, ap, shape, dt):
                t = persist.tile(list(shape), dt, tag=name)
                nc.sync.dma_start(out=t[:], in_=ap[:])
                return t

            idx_sb = load("idx_sb", idx16, [128, 8 * T_tot], I16)
            dstloc_sb = load("dstloc_sb", dstloc, [128, T_tot], BF16)
            poolid_sb = load("poolid_sb", poolid, [128, NB], F32)
            dinvnm_sb = load("dinvnm_sb", dinv_nm, [128, NB], F32)
            dinv2nm_sb = load("dinv2nm_sb", dinv2_nm, [128, NB], F32)
            iota3_sb = load("iota3_sb", iota3, [128, TBBMAX, 128], BF16)
            iotag_sb = load("iotag_sb", iotag, [128, G], BF16)
            identf_sb = load("identf_sb", ident_f, [128, 128], F32)
            identbf_sb = load("identbf_sb", ident_bf, [128, 128], BF16)
            wemb_sb = load("wemb_sb", wemb, [F, H], F32)
            bemb_sb = load("bemb_sb", bemb, [H, 1], F32)
            invc_sb = load("invc_sb", invc, [64, G], F32)
            w1_sb = load("w1_sb", w1, [H, H], F32)
            b1_sb = load("b1_sb", b1, [H, 1], F32)
            w2_sb = load("w2_sb", w2, [H, H // 2], F32)
            b2_sb = load("b2_sb", b2, [H // 2, 1], F32)
            w3_sb = load("w3_sb", w3, [H // 2, 1], F32)
            b3_sb = load("b3_sb", b3, [1, 1], F32)
            cwa_sb = [load(f"cwa{i}_sb", cwa[i], [H + 1, H], F32)
                      for i in range(L)]

            # hagg rows 0:64 = working activation (S or q), row 64 = rdinv
            hagg = persist.tile([H + 1, NS], F32, tag="hagg")
            nc.sync.dma_start(out=hagg[H:H + 1, :], in_=rdinv[:])
            # node-major bf16 copy of this core's scaled shard (self-loops)
            shard_nm = persist.tile([128, NB * H], BF16, tag="shard_nm")
            h3n = persist.tile([128, NB * H], BF16, tag="h3n")

            iota3v = iota3_sb

            def chunks():
                for ci in range(NCH):
                    c0 = ci * CH
                    yield c0, min(CH, NS - c0)

            def table_write(table_out, scale_sb):
                for b in range(NB):
                    w = min(128, NS - b * 128)
                    tp = tr_ps.tile([128, 64], F32, tag="trp")
                    nc.tensor.transpose(
                        out=tp[:w, :], in_=hagg[:H, b * 128:b * 128 + w],
                        identity=identf_sb[:64, :64])
                    nc.vector.tensor_scalar(
                        out=shard_nm[:w, b * H:b * H + H], in0=tp[:w, :],
                        scalar1=scale_sb[:w, b:b + 1], scalar2=None,
                        op0=mybir.AluOpType.mult)
                    nc.sync.dma_start(
                        out=bounce[b * 128:b * 128 + w, 0:H],
                        in_=shard_nm[:w, b * H:b * H + H])
                if variant == "nocoll":
                    nc.sync.dma_start(out=table_out[:NS, :], in_=bounce[:])
                else:
                    nc.gpsimd.collective_compute(
                        "AllGather", mybir.AluOpType.bypass,
                        replica_groups=groups,
                        ins=[bounce[:]], outs=[table_out[:]])

            for _rep in range(repeats):
                # ================= embed =================
                for c0, cwd in chunks():
                    xt = xpool.tile([F, CH], F32, tag="xt")
                    nc.sync.dma_start(out=xt[:, :cwd], in_=x_t[:, c0:c0 + cwd])
                    ps = mm_ps.tile([64, CH], F32, tag="mmps")
                    nc.tensor.matmul(out=ps[:, :cwd], lhsT=wemb_sb[:],
                                     rhs=xt[:, :cwd], start=True, stop=True)
                    nc.scalar.activation(out=hagg[:H, c0:c0 + cwd], in_=ps[:, :cwd],
                                         func=mybir.ActivationFunctionType.Relu,
                                         bias=bemb_sb[:, 0:1])
                table_write(table_a, dinvnm_sb)

                # ================= conv layers =================
                tables = [table_a, table_b, table_a]
                for li in range(L):
                    t_in = tables[li]
                    # emit gathers lazily as block-group sweeps reach them;
                    # g_of maps global tile -> (g tile AP, local offset)
                    g_handles = {}
                    next_instr = [0]

                    def emit_gathers_upto(tmax, t_in=t_in,
                                          g_handles=g_handles,
                                          next_instr=next_instr):
                        while (next_instr[0] < len(instrs)
                               and instrs[next_instr[0]][1] < tmax):
                            ii = next_instr[0]
                            kk, t0, t1 = instrs[ii]
                            nt = t1 - t0
                            g = gpool.tile([128, 8, 128], BF16, tag="g")
                            gi = nc.gpsimd.dma_gather(
                                out_ap=g[:, :nt, :],
                                in_ap=t_in[kk * CS:(kk + 1) * CS, :],
                                idxs_ap=idx_sb[:, 8 * t0:8 * t1],
                                num_idxs=nt * 128, num_idxs_reg=nt * 128,
                                elem_size=128,
                                queue_num=(ii // QSEG) % NQ)
                            for t in range(t0, t1):
                                g_handles[t] = (g, t - t0)
                            next_instr[0] = ii + 1

                    for k in range(NCHK):
                        for b0g in range(0, NB, GB):
                            b1g = min(b0g + GB, NB)
                            T0g = toff_cb[k * NB + b0g]
                            T1g = toff_cb[k * NB + b1g]
                            ng = T1g - T0g
                            emit_gathers_upto(T1g)
                            if ng > 0:
                                oh = ohpool.tile([128, TBBMAX, 128], BF16,
                                                 tag="oh")
                                nc.vector.tensor_tensor(
                                    out=oh[:, :ng, :], in0=iota3v[:, :ng, :],
                                    in1=dstloc_sb[:, T0g:T1g]
                                    .to_broadcast([128, ng, 128]),
                                    op=mybir.AluOpType.is_equal)
                            for b in range(b0g, b1g):
                                w = min(128, NS - b * 128)
                                nt_b = tcb[k][b]
                                if nt_b == 0 and k < NCHK - 1:
                                    continue
                                ps = bp_ps.tile([64, 128], F32, tag="bps")
                                tb0 = toff_cb[k * NB + b]
                                for t in range(tb0, tb0 + nt_b):
                                    g, loc = g_handles[t]
                                    last = (t == tb0 + nt_b - 1
                                            and k < NCHK - 1)
                                    nc.tensor.matmul(
                                        out=ps[:],
                                        lhsT=g[:, loc, 0:H],
                                        rhs=oh[:, t - T0g, :],
                                        start=(t == tb0), stop=last)
                                if k == NCHK - 1:
                                    # self-loop: own scaled shard rows
                                    nc.tensor.matmul(
                                        out=ps[:],
                                        lhsT=shard_nm[:w, b * H:b * H + H],
                                        rhs=identbf_sb[:w, :],
                                        start=(nt_b == 0), stop=True)
                                cols = slice(b * 128, b * 128 + w)
                                if k == firstk[b]:
                                    nc.scalar.activation(
                                        out=hagg[:H, cols], in_=ps[:, :w],
                                        func=mybir.ActivationFunctionType.Copy)
                                else:
                                    nc.vector.tensor_tensor(
                                        out=hagg[:H, cols],
                                        in0=hagg[:H, cols], in1=ps[:, :w],
                                        op=mybir.AluOpType.add)
                    for c0, cwd in chunks():
                        ps = mm_ps.tile([64, CH], F32, tag="mmps")
                        nc.tensor.matmul(out=ps[:, :cwd], lhsT=cwa_sb[li][:],
                                         rhs=hagg[:, c0:c0 + cwd],
                                         start=True, stop=True)
                        nc.scalar.activation(out=hagg[:H, c0:c0 + cwd],
                                             in_=ps[:, :cwd],
                                             func=mybir.ActivationFunctionType.Relu)
                    if li < L - 1:
                        table_write(tables[li + 1], dinv2nm_sb)

                # ================= pooling =================
                # h3 = dinv * q3; pooled = segsum_G(h3)
                for b in range(NB):
                    w = min(128, NS - b * 128)
                    tp = tr_ps.tile([128, 64], F32, tag="trp")
                    nc.tensor.transpose(out=tp[:w, :],
                                        in_=hagg[:H, b * 128:b * 128 + w],
                                        identity=identf_sb[:64, :64])
                    nc.vector.tensor_scalar(
                        out=h3n[:w, b * H:b * H + H], in0=tp[:w, :],
                        scalar1=dinvnm_sb[:w, b:b + 1], scalar2=None,
                        op0=mybir.AluOpType.mult)
                with tc.tile_pool(name="pool_ps", bufs=1, space="PSUM") as pool_ps:
                    pps = pool_ps.tile([64, G], F32, tag="pps")
                    for b in range(NB):
                        w = min(128, NS - b * 128)
                        ohp = pohpool.tile([128, G], BF16, tag="ohp")
                        nc.vector.tensor_scalar(
                            out=ohp[:w, :], in0=iotag_sb[:w, :],
                            scalar1=poolid_sb[:w, b:b + 1], scalar2=None,
                            op0=mybir.AluOpType.is_equal)
                        nc.tensor.matmul(out=pps[:],
                                         lhsT=h3n[:w, b * H:b * H + H],
                                         rhs=ohp[:w, :],
                                         start=(b == 0), stop=(b == NB - 1))
                    psum_sb = persist.tile([64, G], F32, tag="psum_sb")
                    nc.vector.tensor_copy(out=psum_sb[:], in_=pps[:])
                nc.sync.dma_start(out=pool_in[:], in_=psum_sb[:])
                nc.gpsimd.collective_compute(
                    "AllReduce", mybir.AluOpType.add, replica_groups=groups,
                    ins=[pool_in[:]], outs=[pool_out[:]])
                pooled = persist.tile([64, G], F32, tag="pooled")
                nc.sync.dma_start(out=pooled[:], in_=pool_out[:])
                nc.vector.tensor_tensor(out=pooled[:], in0=pooled[:], in1=invc_sb[:],
                                        op=mybir.AluOpType.mult)
                # ================= MLP =================
                ps1 = mm_ps.tile([64, CH], F32, tag="mmps")
                nc.tensor.matmul(out=ps1[:, :G], lhsT=w1_sb[:], rhs=pooled[:],
                                 start=True, stop=True)
                r1 = persist.tile([64, G], F32, tag="r1")
                nc.scalar.activation(out=r1[:], in_=ps1[:64, :G],
                                     func=mybir.ActivationFunctionType.Relu,
                                     bias=b1_sb[:, 0:1])
                ps2 = mm_ps.tile([64, CH], F32, tag="mmps")
                nc.tensor.matmul(out=ps2[:32, :G], lhsT=w2_sb[:], rhs=r1[:],
                                 start=True, stop=True)
                r2 = persist.tile([32, G], F32, tag="r2")
                nc.scalar.activation(out=r2[:], in_=ps2[:32, :G],
                                     func=mybir.ActivationFunctionType.Relu,
                                     bias=b2_sb[:, 0:1])
                ps3 = mm_ps.tile([64, CH], F32, tag="mmps")
                nc.tensor.matmul(out=ps3[:1, :G], lhsT=w3_sb[:], rhs=r2[:],
                                 start=True, stop=True)
                outs = persist.tile([1, G], F32, tag="outs")
                nc.vector.tensor_scalar(out=outs[:], in0=ps3[:1, :G],
                                        scalar1=b3_sb[0:1, 0:1], scalar2=None,
                                        op0=mybir.AluOpType.add)
                nc.sync.dma_start(out=out_d[:], in_=outs[:])

    nc.compile()
    return nc


class SpmdRunner:
    def __init__(self, nc, n_cores):
        install_neuronx_cc_hook()
        self.nc = nc
        self.n_cores = n_cores
        partition_name = (nc.partition_id_tensor.name
                          if nc.partition_id_tensor else None)
        in_names, out_names, out_avals, zero_outs = [], [], [], []
        for alloc in nc.m.functions[0].allocations:
            if not isinstance(alloc, mybir.MemoryLocationSet):
                continue
            name = alloc.memorylocations[0].name
            if alloc.kind == "ExternalInput":
                if name != partition_name:
                    in_names.append(name)
            elif alloc.kind == "ExternalOutput":
                shape = tuple(alloc.tensor_shape)
                dt = mybir.dt.np(alloc.dtype)
                out_names.append(name)
                out_avals.append(jax.core.ShapedArray(shape, dt))
                zero_outs.append(np.zeros(shape, dt))
        self.in_names, self.out_names = in_names, out_names
        self.zero_outs = zero_outs
        bind_in_names = in_names + out_names
        if partition_name is not None:
            bind_in_names.append(partition_name)

        def _body(*args):
            operands = list(args)
            if partition_name is not None:
                operands.append(bass2jax.partition_id_tensor())
            outs = _bass_exec_p.bind(
                *operands,
                out_avals=tuple(out_avals),
                in_names=tuple(bind_in_names),
                out_names=tuple(out_names),
                lowering_input_output_aliases=(),
                sim_require_finite=False,
                sim_require_nnan=False,
                nc=nc,
            )
            return tuple(outs)

        devices = jax.devices()[:n_cores]
        self.mesh = Mesh(np.asarray(devices), ("core",))
        n_args = len(in_names) + len(zero_outs)
        in_specs = (PartitionSpec("core"),) * n_args
        out_specs = (PartitionSpec("core"),) * len(out_names)
        self.fn = jax.jit(
            shard_map(_body, mesh=self.mesh, in_specs=in_specs,
                      out_specs=out_specs, check_rep=False),
            keep_unused=True,
        )
        self._dev_in = None

    def set_inputs(self, in_maps):
        assert len(in_maps) == self.n_cores
        concat = [np.concatenate([np.asarray(in_maps[c][n])
                                  for c in range(self.n_cores)], axis=0)
                  for n in self.in_names]
        self._dev_in = [jax.device_put(a) for a in concat]
        self._dev_zeros = [
            jax.device_put(np.zeros((self.n_cores * z.shape[0], *z.shape[1:]),
                                    z.dtype)) for z in self.zero_outs]
        jax.block_until_ready(self._dev_in)

    def run(self):
        outs = self.fn(*self._dev_in, *self._dev_zeros)
        jax.block_until_ready(outs)
        return outs

    def results(self, outs):
        res = [dict() for _ in range(self.n_cores)]
        for i, name in enumerate(self.out_names):
            arr = np.asarray(outs[i])
            per = np.split(arr, self.n_cores, axis=0)
            for c in range(self.n_cores):
                res[c][name] = per[c]
        return res


_CACHE = {}


def _get_runner(meta, in_maps, repeats=1, variant="full", NQ=4):
    key = (tuple(sorted(meta.__dict__.items())), repeats, variant, NQ)
    if key not in _CACHE:
        nc = build_nc(meta, repeats=repeats, variant=variant, NQ=NQ)
        _CACHE[key] = SpmdRunner(nc, meta.C)
    return _CACHE[key]


def kernel(x, edge_index, batch, W_emb, b_emb, conv_W, conv_b,
           W1, b1, W2, b2, W3, b3):
    """Full (unsharded) inputs -> full [G, 1] float32 output."""
    G = 256
    meta, in_maps = preprocess(
        x, edge_index, batch, W_emb, b_emb, conv_W, conv_b,
        W1, b1, W2, b2, W3, b3, n_cores=8, G=G)
    r = _get_runner(meta, in_maps)
    r.set_inputs(in_maps)
    res = r.results(r.run())
    return np.ascontiguousarray(res[0]["out"].reshape(G, 1).astype(np.float32))

